# revision 1
# baseline (speedup 1.0000x reference)
"""Bass/Tile kernel for nn_DeepseekV3MLPMoEModel on 8 trn2 cores.

Sharding: data-parallel over tokens (T/8 per core) for attention/MLP/lm_head
(vocab-sharded), expert-parallel for the MoE (1 expert/core, dense over all
tokens, ReduceScatter of the weighted sum).

Residual stream layout on device: xT [D(part-chunks of 128), T_loc] (f32r).
"""
import sys
sys.path.insert(0, "/opt/trn_rl_repo")
import numpy as np
import concourse.bass as bass
import concourse.mybir as mybir
import concourse.tile as tile
from concourse import bacc
from concourse.bass_utils import run_bass_kernel_spmd
from concourse.masks import make_identity

F32 = mybir.dt.float32
BF16 = mybir.dt.bfloat16
F32R = mybir.dt.float32r
I32 = mybir.dt.int32
AF = mybir.ActivationFunctionType
OP = mybir.AluOpType
AX = mybir.AxisListType

FULL_CFG = dict(B=2, S=2048, D=1024, H=16, F=2048, E=8, V=32000, L=2, NC=8, G=4)
MINI_CFG = dict(B=2, S=512, D=256, H=4, F=512, E=8, V=1024, L=2, NC=8, G=4)


def derived(cfg):
    c = dict(cfg)
    c["T"] = c["B"] * c["S"]
    c["TC"] = c["T"] // c["NC"]          # tokens per core
    c["TCH"] = c["TC"] // 128            # token tiles per core
    c["DK"] = c["D"] // 128              # D chunks
    c["FK"] = c["F"] // 128              # F chunks
    c["VC"] = c["V"] // c["NC"]          # vocab per core
    c["VCP"] = ((c["VC"] + 127) // 128) * 128
    c["VCK"] = c["VCP"] // 128
    c["VS"] = c["D"] // c["TC"]          # v slots per token-tile in kv pack
    c["SLOTS"] = c["DK"] + c["TCH"] * c["VS"]
    c["dh"] = c["D"] // c["H"]
    assert c["dh"] == 64
    return c


# ---------------------------------------------------------------- host prep

def lhsT_tiles(W, bf16=True):
    """W [M, K] (for out = x @ W.T) -> [M/128, 128(ki), K/128(ko), 128(mm)]."""
    import ml_dtypes
    M, K = W.shape
    Wt = np.ascontiguousarray(W.T)
    r = np.ascontiguousarray(
        Wt.reshape(K // 128, 128, M // 128, 128).transpose(2, 1, 0, 3))
    return r.astype(ml_dtypes.bfloat16) if bf16 else r


def rhs_tiles(W):
    """W [N, K] (used as rhs [K, N]) -> [K/128, 128, N]."""
    N, K = W.shape
    return np.ascontiguousarray(W.T.reshape(K // 128, 128, N))


def pp_cols(b):
    """b [M] -> [128, M/128]: column m holds b[m*128:(m+1)*128]."""
    return np.ascontiguousarray(b.reshape(-1, 128).T)


def prep_in_maps(inputs, cfg):
    c = derived(cfg)
    NC, L, D, E = c["NC"], c["L"], c["D"], c["E"]
    VC, VCP = c["VC"], c["VCP"]
    f32 = np.float32

    tokens = np.asarray(inputs["tokens"]).astype(np.int64).reshape(-1)  # [T]
    emb = np.asarray(inputs["emb"], f32)

    shared = {}
    for l in range(L):
        ipw = np.asarray(inputs["in_proj_w"][l], f32)     # [3D, D]
        ipb = np.asarray(inputs["in_proj_b"][l], f32)     # [3D]
        bqk = ipb[:2 * D].copy()
        bqk[:D] *= 0.125
        shared[f"wqk{l}"] = lhsT_tiles(ipw[:2 * D], bf16=False)
        shared[f"bqk{l}"] = pp_cols(bqk)
        shared[f"wv{l}"] = rhs_tiles(ipw[2 * D:])
        shared[f"bv{l}"] = ipb[2 * D:].reshape(1, D).copy()
        shared[f"wo{l}"] = lhsT_tiles(np.asarray(inputs["out_proj_w"][l], f32), bf16=False)
        shared[f"bo{l}"] = pp_cols(np.asarray(inputs["out_proj_b"][l], f32))
        for nm in ("ln1_w", "ln1_b", "ln2_w", "ln2_b"):
            shared[f"{nm.replace('_','')}{l}"] = pp_cols(np.asarray(inputs[nm][l], f32))
        shared[f"wg{l}"] = lhsT_tiles(np.asarray(inputs["ds_gate_w"][l], f32), bf16=False)
        shared[f"wu{l}"] = lhsT_tiles(np.asarray(inputs["ds_up_w"][l], f32), bf16=False)
        shared[f"wd{l}"] = lhsT_tiles(np.asarray(inputs["ds_down_w"][l], f32), bf16=False)
        shared[f"gw{l}"] = rhs_tiles(np.asarray(inputs["gate_w"][l], f32))
        shared[f"gb{l}"] = np.asarray(inputs["gate_b"][l], f32).reshape(1, E).copy()
    shared["rmsw"] = pp_cols(np.asarray(inputs["rms_w"], f32))
    shared["ones_mat"] = np.ones((128, 128), f32)
    import ml_dtypes
    shared["ones_bf"] = np.ones((128, 64), ml_dtypes.bfloat16)

    in_maps = []
    for core in range(NC):
        m = dict(shared)
        lo = core * VC
        m["embrows"] = emb  # replicated full table
        loc = tokens[core * (len(tokens) // NC):(core + 1) * (len(tokens) // NC)]
        m["tokidx"] = np.ascontiguousarray(
            loc.reshape(-1, 128).T.astype(np.int32))  # [128, TC/128]
        esl = np.zeros((VCP, D), f32)
        esl[:VC] = emb[lo:lo + VC]
        m["embT"] = lhsT_tiles(esl, bf16=c.get("lm_bf16", True))
        for l in range(L):
            m[f"w1{l}"] = lhsT_tiles(np.asarray(inputs["moe_w1"][l, core], f32), bf16=False)
            m[f"b1{l}"] = pp_cols(np.asarray(inputs["moe_b1"][l, core], f32))
            m[f"w2{l}"] = lhsT_tiles(np.asarray(inputs["moe_w2"][l, core], f32), bf16=False)
            m[f"b2{l}"] = pp_cols(np.asarray(inputs["moe_b2"][l, core], f32))
        in_maps.append(m)
    return in_maps


def assemble_logits(results, cfg):
    c = derived(cfg)
    B, S, V, VC = c["B"], c["S"], c["V"], c["VC"]
    out = np.empty((B, S, V), np.float32)
    for core, r in enumerate(results):
        lg = r["logits"]  # [VC, T]
        out[:, :, core * VC:(core + 1) * VC] = lg.T.reshape(B, S, VC)
    return out


# ---------------------------------------------------------------- device code

def build_nc(cfg):
    c = derived(cfg)
    L, D, E = c["L"], c["D"], c["E"]
    DK, FK = c["DK"], c["FK"]
    VC, VCK = c["VC"], c["VCK"]
    T = c["T"]

    nc = bacc.Bacc(None)
    P = {}

    def par(name, shape, dt):
        P[name] = nc.dram_tensor(name, shape, dt, kind="ExternalInput")

    par("tokidx", [128, T // (8 * 128)], I32)
    par("ones_mat", [128, 128], F32R)
    par("ones_bf", [128, 64], BF16)
    par("embrows", [c["V"], D], F32)
    par("embT", [VCK, 128, DK, 128], BF16 if c.get("lm_bf16", True) else F32R)
    for l in range(L):
        par(f"wqk{l}", [2 * DK, 128, DK, 128], F32R)
        par(f"bqk{l}", [128, 2 * DK], F32)
        par(f"wv{l}", [DK, 128, D], F32R)
        par(f"bv{l}", [1, D], F32R)
        par(f"wo{l}", [DK, 128, DK, 128], F32R)
        par(f"bo{l}", [128, DK], F32)
        for nm in ("ln1w", "ln1b", "ln2w", "ln2b"):
            par(f"{nm}{l}", [128, DK], F32)
        par(f"wg{l}", [FK, 128, DK, 128], F32R)
        par(f"wu{l}", [FK, 128, DK, 128], F32R)
        par(f"wd{l}", [DK, 128, FK, 128], F32R)
        par(f"gw{l}", [DK, 128, E], F32)
        par(f"gb{l}", [1, E], F32R)
        par(f"w1{l}", [FK, 128, DK, 128], F32R)
        par(f"b1{l}", [128, FK], F32)
        par(f"w2{l}", [DK, 128, FK, 128], F32R)
        par(f"b2{l}", [128, DK], F32)
    par("rmsw", [128, DK], F32)
    OUT = nc.dram_tensor("logits", [VC, T], F32, kind="ExternalOutput")

    with tile.TileContext(nc) as tc:
        _emit(nc, tc, P, OUT, c)
    nc.compile()
    return nc


def _emit(nc, tc, P, OUT, c):
    NC, L, D, H, F, E = c["NC"], c["L"], c["D"], c["H"], c["F"], c["E"]
    TC, TCH, DK, FK = c["TC"], c["TCH"], c["DK"], c["FK"]
    VC, VCK, VS, SLOTS = c["VC"], c["VCK"], c["VS"], c["SLOTS"]
    G, T = c["G"], c["T"]
    KCH = G * TCH
    TK = T // 128
    NDN = max(1, D // 512)
    NW = min(512, D)
    GRP_KV = [list(range(g * G, (g + 1) * G)) for g in range(NC // G)]
    GRP_ALL = [list(range(NC))]

    from contextlib import ExitStack
    es = ExitStack()
    cst = es.enter_context(tc.tile_pool(name="cst", bufs=1))
    sbt = es.enter_context(tc.tile_pool(name="sbt", bufs=2))
    lnp = es.enter_context(tc.tile_pool(name="lnp", bufs=2))
    xlp = es.enter_context(tc.tile_pool(name="xlp", bufs=1))
    psm = es.enter_context(tc.tile_pool(name="psm", bufs=4, space="PSUM"))
    pst = es.enter_context(tc.tile_pool(name="pst", bufs=2, space="PSUM"))
    ptr = es.enter_context(tc.tile_pool(name="ptr", bufs=2, space="PSUM"))
    drp = es.enter_context(tc.tile_pool(name="drp", bufs=1, space="DRAM"))

    dbg_on = c.get("debug", False)

    def dbg(name, ap):
        if not dbg_on:
            return
        t = nc.dram_tensor(f"dbg_{name}", list(ap.shape), ap.dtype,
                           kind="ExternalOutput")
        nc.sync.dma_start(t[:], ap)

    ident = cst.tile([128, 128], F32, name="ident")
    make_identity(nc, ident)
    ones_m = cst.tile([128, 128], F32R, name="ones_m")
    nc.sync.dma_start(ones_m[:], P["ones_mat"][:])
    eps5 = cst.tile([128, 1], F32, name="eps5")
    nc.gpsimd.memset(eps5[:], 1e-5)
    eps6 = cst.tile([128, 1], F32, name="eps6")
    nc.gpsimd.memset(eps6[:], 1e-6)
    xT = cst.tile([128, DK, TC], F32R, name="xT")
    
    KCH_ = G * TCH


    # ---------------- embedding: gather own tokens from replicated table
    with tc.tile_pool(name="emb_ph", bufs=3) as ph:
        idx_sb = ph.tile([128, TCH], I32, name="idx_sb", bufs=1)
        nc.sync.dma_start(idx_sb[:], P["tokidx"][:])
        sqrt_d = float(np.sqrt(c["D"]))
        for tm in range(TCH):
            ge = ph.tile([128, D], F32, tag="ge")
            nc.gpsimd.indirect_dma_start(
                out=ge[:], out_offset=None, in_=P["embrows"][:],
                in_offset=bass.IndirectOffsetOnAxis(ap=idx_sb[:, tm:tm + 1], axis=0))
            for k in range(DK):
                pt = ptr.tile([128, 128], F32, tag="ptr")
                nc.tensor.transpose(pt[:], ge[:, k * 128:(k + 1) * 128], ident[:])
                nc.scalar.activation(xT[:, k, tm * 128:(tm + 1) * 128], pt[:],
                                     AF.Copy, scale=sqrt_d)
    dbg("x0T", xT[:])

    # ---------------- LN helper (matmul stats, replicated across partitions)
    def layer_norm_(dst, src, wcols, bcols, eps, skip_mean=False):
        eps = eps5[:, 0:1] if eps == 1e-5 else eps6[:, 0:1]
        ps1 = None if skip_mean else pst.tile([128, TC], F32, tag="pstat")
        ps2 = pst.tile([128, TC], F32, tag="pstat")
        for k in range(DK):
            sq = lnp.tile([128, TC], F32R, tag="sq")
            nc.vector.tensor_tensor(sq[:], src[:, k, :], src[:, k, :], OP.mult)
            if not skip_mean:
                nc.tensor.matmul(ps1[:], ones_m[:], src[:, k, :],
                                 start=(k == 0), stop=(k == DK - 1))
            nc.tensor.matmul(ps2[:], ones_m[:], sq[:],
                             start=(k == 0), stop=(k == DK - 1))
        e2 = lnp.tile([128, TC], F32, tag="stmp")
        nc.scalar.activation(e2[:], ps2[:], AF.Copy, scale=1.0 / c["D"])
        if not skip_mean:
            mu = lnp.tile([128, TC], F32, tag="smu", bufs=1)
            nc.scalar.activation(mu[:], ps1[:], AF.Copy, scale=1.0 / c["D"])
            var = lnp.tile([128, TC], F32, tag="stmp")
            nc.vector.tensor_tensor(var[:], mu[:], mu[:], OP.mult)
            nc.vector.tensor_tensor(var[:], e2[:], var[:], OP.subtract)
        else:
            var = e2
        sd = lnp.tile([128, TC], F32, tag="stmp")
        nc.scalar.activation(sd[:], var[:], AF.Sqrt, bias=eps)
        rstd = lnp.tile([128, TC], F32, tag="srstd", bufs=1)
        nc.vector.reciprocal(rstd[:], sd[:])
        for k in range(DK):
            t1 = lnp.tile([128, TC], F32, tag="lnt")
            if not skip_mean:
                nc.vector.tensor_tensor(t1[:], src[:, k, :], mu[:], OP.subtract)
                nc.vector.tensor_tensor(t1[:], t1[:], rstd[:], OP.mult)
            else:
                nc.vector.tensor_tensor(t1[:], src[:, k, :], rstd[:], OP.mult)
            if bcols is not None:
                nc.vector.tensor_scalar(dst[:, k, :], t1[:],
                                        wcols[:, k:k + 1], bcols[:, k:k + 1],
                                        OP.mult, OP.add)
            else:
                nc.vector.tensor_scalar_mul(dst[:, k, :], t1[:], wcols[:, k:k + 1])

    # ---------------- layers
    for l in range(L):
        lb = {}
        for nm in ("bqk", "bo", "ln1w", "ln1b", "ln2w", "ln2b", "b1", "b2"):
            w = P[f"{nm}{l}"].shape[1]
            t = cst.tile([128, w], F32, name=f"{nm}{l}_sb", tag=f"c_{nm}")
            nc.sync.dma_start(t[:], P[f"{nm}{l}"][:])
            lb[nm] = t
        bv1 = cst.tile([1, D], F32R, name=f"bv1_{l}", tag="c_bv1")
        nc.sync.dma_start(bv1[:], P[f"bv{l}"][:])
        bv = cst.tile([128, D], F32, name=f"bv{l}_sb", tag="c_bv")
        for dn in range(NDN):
            psb = psm.tile([128, NW], F32, tag="psmm")
            nc.tensor.matmul(psb[:], ones_m[0:1, :],
                             bv1[0:1, dn * NW:(dn + 1) * NW], start=True, stop=True)
            nc.vector.tensor_copy(bv[:, dn * NW:(dn + 1) * NW], psb[:])
        gb1 = cst.tile([1, E], F32R, name=f"gb1_{l}", tag="c_gb1")
        nc.sync.dma_start(gb1[:], P[f"gb{l}"][:])
        psgb = psm.tile([128, E], F32, tag="psmm")
        nc.tensor.matmul(psgb[:], ones_m[0:1, :], gb1[0:1, :], start=True, stop=True)
        gb = cst.tile([128, E], F32, name=f"gb{l}_sb", tag="c_gb")
        nc.vector.tensor_copy(gb[:], psgb[:])

        kv_in = [drp.tile([128, TC], F32R, name=f"kvin{sl}", tag=f"kvin{sl}")
                 for sl in range(SLOTS)]
        kv_all = [drp.tile([G, 128, TC], F32R, name=f"kvall{sl}", tag=f"kvall{sl}")
                  for sl in range(SLOTS)]

        # --- qkv phase (v first, then q/k with chunked k gathers)
        with tc.tile_pool(name="qp", bufs=1) as qp:
            q_sb = qp.tile([128, DK, TC], F32R, tag="q_sb")
            with (
                tc.tile_pool(name="qphw", bufs=4) as qphw,
                tc.tile_pool(name="qphk", bufs=2) as qphk,
                tc.tile_pool(name="qpv", bufs=1) as qpv,
            ):
                for dn in range(NDN):
                    wv = qpv.tile([128, DK, NW], F32R, tag="wv")
                    for k in range(DK):
                        nc.sync.dma_start(wv[:, k, :],
                                          P[f"wv{l}"][k, :, dn * NW:(dn + 1) * NW])
                    for tm in range(TCH):
                        ps = psm.tile([128, NW], F32, tag="psmm")
                        for k in range(DK):
                            nc.tensor.matmul(ps[:], xT[:, k, tm * 128:(tm + 1) * 128],
                                             wv[:, k, :],
                                             start=(k == 0), stop=(k == DK - 1))
                        vt = qphk.tile([128, NW], F32R, tag="vt")
                        nc.vector.tensor_tensor(
                            vt[:], ps[:], bv[:, dn * NW:(dn + 1) * NW], OP.add)
                        for sl in range(max(1, NW // TC)):
                            w_ = min(TC, NW)
                            slot = DK + tm * VS + (dn * NW) // TC + sl
                            nc.sync.dma_start(kv_in[slot][:],
                                              vt[:, sl * w_:(sl + 1) * w_])
                            nc.gpsimd.collective_compute(
                                "AllGather", OP.bypass, replica_groups=GRP_KV,
                                ins=[kv_in[slot][:]], outs=[kv_all[slot][:]])
                for m in range(2 * DK):
                    wt = qphw.tile([128, DK, 128], F32R, tag="wt")
                    nc.sync.dma_start(wt[:], P[f"wqk{l}"][m])
                    ps = psm.tile([128, TC], F32, tag="psmm")
                    for k in range(DK):
                        nc.tensor.matmul(ps[:], wt[:, k, :], xT[:, k, :],
                                         start=(k == 0), stop=(k == DK - 1))
                    if m < DK:
                        nc.scalar.activation(q_sb[:, m, :], ps[:], AF.Identity,
                                             scale=0.125, bias=lb["bqk"][:, m:m + 1])
                    else:
                        kt = qphk.tile([128, TC], F32R, tag="kt")
                        nc.scalar.activation(kt[:], ps[:], AF.Identity,
                                             bias=lb["bqk"][:, m:m + 1])
                        nc.sync.dma_start(kv_in[m - DK][:], kt[:])
                        nc.gpsimd.collective_compute(
                            "AllGather", OP.bypass, replica_groups=GRP_KV,
                            ins=[kv_in[m - DK][:]], outs=[kv_all[m - DK][:]])
            if l == 0:
                dbg("q0", q_sb[:])

            # --- attention (q_sb in scope)
            with tc.tile_pool(name="aoT", bufs=1) as aoTp:
                oT = aoTp.tile([128, DK, TC], F32R, tag="oT")
                vh2 = aoTp.tile([128, 2, KCH, 128], F32R, tag="vh2")
                for b_ in range(2):
                    for kc_ in range(KCH):
                        nc.sync.dma_start(vh2[:, b_, kc_, 64:128],
                                          P["ones_mat"][:, 0:64])
                with (
                    tc.tile_pool(name="aph", bufs=2) as aph,
                    tc.tile_pool(name="apT", bufs=1) as apTp,
                ):
                    for h in range(H):
                        qm, qoff = h // 2, 64 * (h % 2)
                        kh = aph.tile([128, G, TC], F32R, tag="kh")
                        for g in range(G):
                            nc.sync.dma_start(kh[qoff:qoff + 64, g, :],
                                              kv_all[qm][g, qoff:qoff + 64, :])
                        s_v, off_v = (64 * h) // TC, (64 * h) % TC
                        for g in range(G):
                            for tm in range(TCH):
                                nc.sync.dma_start(
                                    vh2[:, h % 2, g * TCH + tm, 0:64],
                                    kv_all[DK + tm * VS + s_v][g, :, off_v:off_v + 64])
                        pT = apTp.tile([128, KCH, TC], F32R, tag="pT")
                        for kc in range(KCH):
                            ps = psm.tile([128, TC], F32, tag="psmm")
                            nc.tensor.matmul(
                                ps[:],
                                kh[qoff:qoff + 64, kc // TCH,
                                   (kc % TCH) * 128:(kc % TCH) * 128 + 128],
                                q_sb[qoff:qoff + 64, qm, :], start=True, stop=True)
                            nc.scalar.activation(pT[:, kc, :], ps[:], AF.Exp)
                        po = psm.tile([128, TC], F32, tag="psmm")
                        for kc in range(KCH):
                            nc.tensor.matmul(po[:], vh2[:, h % 2, kc, :],
                                             pT[:, kc, :],
                                             start=(kc == 0), stop=(kc == KCH - 1))
                        rec = sbt.tile([64, TC], F32, tag="rec")
                        nc.vector.reciprocal(rec[:], po[64:128, :])
                        nc.vector.tensor_tensor(oT[qoff:qoff + 64, qm, :],
                                                po[0:64, :], rec[:], OP.mult)
                if l == 0:
                    dbg("oT0", oT[:])
                # --- out proj + residual + ln1
                with tc.tile_pool(name="oph", bufs=4) as oph:
                    xln = xlp.tile([128, DK, TC], F32R, tag="xln")
                    for m in range(DK):
                        wt = oph.tile([128, DK, 128], F32R, tag="wt")
                        nc.sync.dma_start(wt[:], P[f"wo{l}"][m])
                        ps = psm.tile([128, TC], F32, tag="psmm")
                        for k in range(DK):
                            nc.tensor.matmul(ps[:], wt[:, k, :], oT[:, k, :],
                                             start=(k == 0), stop=(k == DK - 1))
                        t = sbt.tile([128, TC], F32, tag="ot")
                        nc.vector.tensor_scalar_add(t[:], ps[:], lb["bo"][:, m:m + 1])
                        nc.vector.tensor_tensor(xln[:, m, :], t[:], xT[:, m, :],
                                                OP.add)
                    layer_norm_(xT, xln, lb["ln1w"], lb["ln1b"], 1e-5)
        if l == 0:
            dbg("xln1_0", xT[:])

        # --- allgather x for MoE (issued early; overlaps ds)
        xg_in = drp.tile([128, DK, TC], F32R, name="xgin", tag="xgin")
        xg_all = drp.tile([NC, 128, DK, TC], F32R, name="xgall", tag="xgall",
                          addr_space="Shared")
        nc.sync.dma_start(xg_in[:], xT[:])
        nc.gpsimd.collective_compute(
            "AllGather", OP.bypass, replica_groups=GRP_ALL,
            ins=[xg_in[:]], outs=[xg_all[:]])

        # --- router: gate scores for local tokens -> top2 -> AllToAll
        dT_in = drp.tile([E, TC], F32R, name="dtin", tag="dtin")
        d_all = drp.tile([NC, TC], F32R, name="dall", tag="dall")
        with tc.tile_pool(name="rph", bufs=2) as rph:
            gwt = rph.tile([128, DK, E], F32, tag="gwt", bufs=1)
            for k in range(DK):
                nc.sync.dma_start(gwt[:, k, :], P[f"gw{l}"][k])
            for tm in range(TCH):
                xf = rph.tile([128, DK, 128], F32, tag="xf")
                for k in range(DK):
                    nc.vector.tensor_copy(xf[:, k, :],
                                          xT[:, k, tm * 128:(tm + 1) * 128])
                psg = psm.tile([128, E], F32, tag="psmm")
                for k in range(DK):
                    nc.tensor.matmul(psg[:], xf[:, k, :], gwt[:, k, :],
                                     start=(k == 0), stop=(k == DK - 1))
                gs = rph.tile([128, E], F32, tag="gs")
                nc.vector.tensor_tensor(gs[:], psg[:], gb[:], OP.add)
                m1 = rph.tile([128, 1], F32, tag="m1")
                nc.vector.tensor_reduce(m1[:], gs[:], AX.X, OP.max)
                mask1 = rph.tile([128, E], F32, tag="mask1")
                nc.vector.tensor_tensor(mask1[:], gs[:],
                                        m1[:].to_broadcast([128, E]), OP.is_equal)
                gs2 = rph.tile([128, E], F32, tag="gs2")
                nc.vector.tensor_scalar_mul(gs2[:], mask1[:], -1e30)
                nc.vector.tensor_tensor(gs2[:], gs2[:], gs[:], OP.add)
                m2 = rph.tile([128, 1], F32, tag="m2")
                nc.vector.tensor_reduce(m2[:], gs2[:], AX.X, OP.max)
                mask2 = rph.tile([128, E], F32, tag="mask2")
                nc.vector.tensor_tensor(mask2[:], gs2[:],
                                        m2[:].to_broadcast([128, E]), OP.is_equal)
                dm = rph.tile([128, 1], F32, tag="dm")
                nc.vector.tensor_tensor(dm[:], m2[:], m1[:], OP.subtract)
                nc.scalar.activation(dm[:], dm[:], AF.Exp)
                nc.vector.tensor_scalar_add(dm[:], dm[:], 1.0)
                w1t = rph.tile([128, 1], F32, tag="w1t")
                nc.vector.reciprocal(w1t[:], dm[:])
                w2t = rph.tile([128, 1], F32, tag="w2t")
                nc.vector.tensor_scalar(w2t[:], w1t[:], -1.0, 1.0, OP.mult, OP.add)
                dmat = rph.tile([128, E], F32, tag="dmat")
                nc.vector.tensor_scalar_mul(dmat[:], mask1[:], w1t[:, 0:1])
                dm2 = rph.tile([128, E], F32, tag="dm2")
                nc.vector.tensor_scalar_mul(dm2[:], mask2[:], w2t[:, 0:1])
                nc.vector.tensor_tensor(dmat[:], dmat[:], dm2[:], OP.add)
                ptd = ptr.tile([128, 128], F32, tag="ptr")
                nc.tensor.transpose(ptd[0:E, :], dmat[:], ident[:])
                dtt = rph.tile([E, 128], F32R, tag="dtt")
                nc.vector.tensor_copy(dtt[:], ptd[0:E, 0:128])
                nc.sync.dma_start(dT_in[:, tm * 128:(tm + 1) * 128], dtt[:])
        if l == 0:
            dbg("dT0", dT_in[:])
        nc.gpsimd.collective_compute(
            "AllToAll", OP.bypass, replica_groups=GRP_ALL,
            ins=[dT_in[:]], outs=[d_all[:]])

        # --- ds mlp (local tokens)
        dsT_pool = tc.tile_pool(name="dsT", bufs=1)
        dsTp = dsT_pool.__enter__()
        dsT = dsTp.tile([128, DK, TC], F32, tag="dsT")
        with (
            tc.tile_pool(name="dph", bufs=1) as dph,
            tc.tile_pool(name="dphw", bufs=3) as dphw,
        ):
            gu = dph.tile([128, FK, TC], F32R, tag="gu")
            for m in range(FK):
                wtg = dphw.tile([128, DK, 128], F32R, tag="wt")
                nc.sync.dma_start(wtg[:], P[f"wg{l}"][m])
                psg = psm.tile([128, TC], F32, tag="psmm")
                for k in range(DK):
                    nc.tensor.matmul(psg[:], wtg[:, k, :], xT[:, k, :],
                                     start=(k == 0), stop=(k == DK - 1))
                sg = sbt.tile([128, TC], F32, tag="sg")
                nc.scalar.activation(sg[:], psg[:], AF.Sigmoid)
                nc.vector.tensor_tensor(sg[:], sg[:], psg[:], OP.mult)
                wtu = dphw.tile([128, DK, 128], F32R, tag="wt")
                nc.sync.dma_start(wtu[:], P[f"wu{l}"][m])
                psu = psm.tile([128, TC], F32, tag="psmm")
                for k in range(DK):
                    nc.tensor.matmul(psu[:], wtu[:, k, :], xT[:, k, :],
                                     start=(k == 0), stop=(k == DK - 1))
                nc.vector.tensor_tensor(gu[:, m, :], sg[:], psu[:], OP.mult)
            for m in range(DK):
                wtd = dphw.tile([128, FK, 128], F32R, tag="wtd", bufs=2)
                nc.sync.dma_start(wtd[:], P[f"wd{l}"][m])
                psd = psm.tile([128, TC], F32, tag="psmm")
                for k in range(FK):
                    nc.tensor.matmul(psd[:], wtd[:, k, :], gu[:, k, :],
                                     start=(k == 0), stop=(k == FK - 1))
                nc.vector.tensor_copy(dsT[:, m, :], psd[:])
        if l == 0:
            dbg("dsT0", dsT[:])

        # --- MoE expert pass over all token chunks
        rs_in = [drp.tile([NC, 128, TC], F32, name=f"rsin{m}", tag=f"rsin{m}")
                 for m in range(DK)]
        rs_out = [drp.tile([128, TC], F32, name=f"rsout{m}", tag=f"rsout{m}")
                  for m in range(DK)]
        with (
            tc.tile_pool(name="mph", bufs=2) as mph,
            tc.tile_pool(name="mphh", bufs=1) as mphh,
            tc.tile_pool(name="mphw", bufs=3) as mphw,
        ):
            for n in range(NC):
                xan = mph.tile([128, DK, TC], F32R, tag="xan")
                nc.sync.dma_start(xan[:], xg_all[n])
                dn_t = sbt.tile([1, TC], F32R, tag="dn")
                nc.sync.dma_start(dn_t[0:1, :], d_all[n:n + 1, :])
                psd = psm.tile([128, TC], F32, tag="psmm")
                nc.tensor.matmul(psd[:], ones_m[0:1, :], dn_t[0:1, :],
                                 start=True, stop=True)
                dnr = sbt.tile([128, TC], F32, tag="dnr")
                nc.vector.tensor_copy(dnr[:], psd[:])
                hT = mphh.tile([128, FK, TC], F32R, tag="hT")
                for m in range(FK):
                    wt1 = mphw.tile([128, DK, 128], F32R, tag="wt")
                    nc.sync.dma_start(wt1[:], P[f"w1{l}"][m])
                    ps = psm.tile([128, TC], F32, tag="psmm")
                    for k in range(DK):
                        nc.tensor.matmul(ps[:], wt1[:, k, :], xan[:, k, :],
                                         start=(k == 0), stop=(k == DK - 1))
                    nc.scalar.activation(hT[:, m, :], ps[:], AF.Relu,
                                         bias=lb["b1"][:, m:m + 1])
                for m in range(DK):
                    wt2 = mphw.tile([128, FK, 128], F32R, tag="wtd", bufs=2)
                    nc.sync.dma_start(wt2[:], P[f"w2{l}"][m])
                    ps = psm.tile([128, TC], F32, tag="psmm")
                    for k in range(FK):
                        nc.tensor.matmul(ps[:], wt2[:, k, :], hT[:, k, :],
                                         start=(k == 0), stop=(k == FK - 1))
                    ew = sbt.tile([128, TC], F32, tag="ew")
                    nc.vector.tensor_scalar_add(ew[:], ps[:], lb["b2"][:, m:m + 1])
                    nc.vector.tensor_tensor(ew[:], ew[:], dnr[:], OP.mult)
                    nc.sync.dma_start(rs_in[m][n], ew[:])
                    if n == NC - 1:
                        nc.gpsimd.collective_compute(
                            "ReduceScatter", OP.add, replica_groups=GRP_ALL,
                            ins=[rs_in[m][:]], outs=[rs_out[m][:]])


        # --- combine + ln2
        xln2 = xlp.tile([128, DK, TC], F32R, tag="xln")
        for m in range(DK):
            mo = sbt.tile([128, TC], F32, tag="mo")
            nc.sync.dma_start(mo[:], rs_out[m][:])
            nc.vector.tensor_tensor(mo[:], mo[:], dsT[:, m, :], OP.add)
            nc.vector.tensor_scalar_mul(mo[:], mo[:], 0.5)
            nc.vector.tensor_tensor(xln2[:, m, :], mo[:], xT[:, m, :], OP.add)
        layer_norm_(xT, xln2, lb["ln2w"], lb["ln2b"], 1e-5)
        dsT_pool.__exit__(None, None, None)

    dbg("xfinal", xT[:])
    # ---------------- final rms + allgather + lm_head
    rmsw = cst.tile([128, DK], F32, name="rmsw_sb")
    nc.sync.dma_start(rmsw[:], P["rmsw"][:])
    LMDT0 = BF16 if c.get("lm_bf16", True) else F32R
    xf_in = drp.tile([128, DK, TC], LMDT0, name="xfin")
    xf_all = drp.tile([NC, 128, DK, TC], LMDT0, name="xfall", addr_space="Shared")
    xr = xlp.tile([128, DK, TC], F32R, tag="xln")
    layer_norm_(xr, xT, rmsw, None, 1e-6, skip_mean=True)
    LMDT = BF16 if c.get("lm_bf16", True) else F32R
    xrb = xlp.tile([128, DK, TC], LMDT, tag="xrb")
    for k in range(DK):
        nc.vector.tensor_copy(xrb[:, k, :], xr[:, k, :])
    nc.sync.dma_start(xf_in[:], xrb[:])
    nc.gpsimd.collective_compute(
        "AllGather", OP.bypass, replica_groups=GRP_ALL,
        ins=[xf_in[:]], outs=[xf_all[:]])
    with (
        tc.tile_pool(name="lph", bufs=2) as lph,
        tc.tile_pool(name="lphw", bufs=4) as lphw,
    ):
        for n in range(NC):
            xfn = lph.tile([128, DK, TC], LMDT0, tag="xan")
            nc.sync.dma_start(xfn[:], xf_all[n])
            for m in range(VCK):
                wt = lphw.tile([128, DK, 128], LMDT0, tag="wt")
                nc.sync.dma_start(wt[:], P["embT"][m])
                ps = psm.tile([128, TC], F32, tag="psmm")
                for k in range(DK):
                    nc.tensor.matmul(ps[:], wt[:, k, :], xfn[:, k, :],
                                     start=(k == 0), stop=(k == DK - 1))
                lo = sbt.tile([128, TC], F32, tag="lo")
                nc.vector.tensor_copy(lo[:], ps[:])
                rows = min(128, VC - m * 128)
                nc.sync.dma_start(
                    OUT[m * 128:m * 128 + rows, n * TC:(n + 1) * TC], lo[:rows, :])

    es.close()


# ---------------------------------------------------------------- runner

def run_model(inputs, cfg, nc=None):
    c = derived(cfg)
    in_maps = prep_in_maps(inputs, cfg)
    if nc is None:
        nc = build_nc(cfg)
    res = run_bass_kernel_spmd(nc, in_maps, core_ids=list(range(c["NC"])))
    return assemble_logits(res.results, cfg), nc


# ---------------------------------------------------------------- entry point

_NC_CACHE = None


def kernel(**inputs):
    """Full-model forward on 8 trn2 cores. inputs as in reference.setup_inputs()."""
    global _NC_CACHE
    import numpy as _np
    inputs = {k: _np.asarray(v) for k, v in inputs.items()}
    if _NC_CACHE is None:
        _NC_CACHE = build_nc(FULL_CFG)
    in_maps = prep_in_maps(inputs, FULL_CFG)
    res = run_bass_kernel_spmd(_NC_CACHE, in_maps,
                               core_ids=list(range(FULL_CFG["NC"])))
    return assemble_logits(res.results, FULL_CFG)



# revision 18
# speedup vs baseline: 1.4054x; 1.4054x over previous
"""Bass/Tile kernel for nn_DeepseekV3MLPMoEModel on 8 trn2 cores.

Sharding: data-parallel over tokens (T/8 per core) for attention/MLP/lm_head
(vocab-sharded), expert-parallel for the MoE with TRUE top-2 routing:
each core owns one expert; owner cores scatter their routed token rows into a
per-expert staging buffer (capacity CLOC per core-expert pair), AllToAll
dispatches rows to the expert cores, each expert runs its dense pass over its
(<= NC*CLOC) rows only, AllToAll returns the outputs, and owner cores gather
their two rows per token and apply the top-2 softmax weights.

Residual stream layout on device: xT [D(part-chunks of 128), T_loc] (f32r).
"""
import sys
sys.path.insert(0, "/opt/trn_rl_repo")
import numpy as np
import concourse.bass as bass
import concourse.mybir as mybir
import concourse.tile as tile
from concourse import bacc
from concourse.bass_utils import run_bass_kernel_spmd
from concourse.masks import make_identity

F32 = mybir.dt.float32
BF16 = mybir.dt.bfloat16
F32R = mybir.dt.float32r
I32 = mybir.dt.int32
AF = mybir.ActivationFunctionType
OP = mybir.AluOpType
AX = mybir.AxisListType

FULL_CFG = dict(B=2, S=2048, D=1024, H=16, F=2048, E=8, V=32000, L=2, NC=8, G=4,
                CLOC=192)


def derived(cfg):
    c = dict(cfg)
    c["T"] = c["B"] * c["S"]
    c["TC"] = c["T"] // c["NC"]          # tokens per core
    c["TCH"] = c["TC"] // 128            # token tiles per core
    c["DK"] = c["D"] // 128              # D chunks
    c["FK"] = c["F"] // 128              # F chunks
    c["VC"] = c["V"] // c["NC"]          # vocab per core
    c["VCP"] = ((c["VC"] + 127) // 128) * 128
    c["VCK"] = c["VCP"] // 128
    c["VS"] = c["D"] // c["TC"]          # v slots per token-tile in kv pack
    c["SLOTS"] = c["DK"] + c["TCH"] * c["VS"]
    c["dh"] = c["D"] // c["H"]
    c["RSTG"] = c["E"] * c["CLOC"]       # expert staging rows
    assert c["RSTG"] % 512 == 0
    c["CCH"] = c["RSTG"] // 512          # expert column chunks of 512 rows
    assert c["dh"] == 64
    return c


# ---------------------------------------------------------------- host prep

def lhsT_tiles(W, bf16=True):
    """W [M, K] (for out = x @ W.T) -> [M/128, 128(ki), K/128(ko), 128(mm)]."""
    import ml_dtypes
    M, K = W.shape
    Wt = np.ascontiguousarray(W.T)
    r = np.ascontiguousarray(
        Wt.reshape(K // 128, 128, M // 128, 128).transpose(2, 1, 0, 3))
    return r.astype(ml_dtypes.bfloat16) if bf16 else r


def rhs_tiles(W, bf16=False):
    """W [N, K] (used as rhs [K, N]) -> [K/128, 128, N]."""
    import ml_dtypes
    N, K = W.shape
    r = np.ascontiguousarray(W.T.reshape(K // 128, 128, N))
    return r.astype(ml_dtypes.bfloat16) if bf16 else r


def pp_cols(b):
    """b [M] -> [128, M/128]: column m holds b[m*128:(m+1)*128]."""
    return np.ascontiguousarray(b.reshape(-1, 128).T)


def prep_in_maps(inputs, cfg):
    c = derived(cfg)
    NC, L, D, E = c["NC"], c["L"], c["D"], c["E"]
    VC, VCP = c["VC"], c["VCP"]
    f32 = np.float32

    tokens = np.asarray(inputs["tokens"]).astype(np.int64).reshape(-1)  # [T]
    emb = np.asarray(inputs["emb"], f32)

    shared = {}
    for l in range(L):
        ipw = np.asarray(inputs["in_proj_w"][l], f32)     # [3D, D]
        ipb = np.asarray(inputs["in_proj_b"][l], f32)     # [3D]
        bqk = ipb[:2 * D].copy()
        bqk[:D] *= 0.125
        shared[f"wqk{l}"] = lhsT_tiles(ipw[:2 * D], bf16=False)
        shared[f"bqk{l}"] = pp_cols(bqk)
        shared[f"wv{l}"] = rhs_tiles(ipw[2 * D:])
        shared[f"bv{l}"] = ipb[2 * D:].reshape(1, D).copy()
        shared[f"wo{l}"] = lhsT_tiles(np.asarray(inputs["out_proj_w"][l], f32), bf16=False)
        shared[f"bo{l}"] = pp_cols(np.asarray(inputs["out_proj_b"][l], f32))
        for nm in ("ln1_w", "ln1_b", "ln2_w", "ln2_b"):
            shared[f"{nm.replace('_','')}{l}"] = pp_cols(np.asarray(inputs[nm][l], f32))
        shared[f"wg{l}"] = lhsT_tiles(np.asarray(inputs["ds_gate_w"][l], f32), bf16=False)
        shared[f"wu{l}"] = lhsT_tiles(np.asarray(inputs["ds_up_w"][l], f32), bf16=False)
        shared[f"wd{l}"] = lhsT_tiles(np.asarray(inputs["ds_down_w"][l], f32), bf16=False)
        shared[f"gw{l}"] = rhs_tiles(np.asarray(inputs["gate_w"][l], f32))
        shared[f"gb{l}"] = np.asarray(inputs["gate_b"][l], f32).reshape(1, E).copy()
    shared["rmsw"] = pp_cols(np.asarray(inputs["rms_w"], f32))
    shared["ones_mat"] = np.ones((128, 128), f32)
    shared["lt_mat"] = np.triu(np.ones((128, 128), f32))  # lt[k,p]=1 iff k<=p
    shared["ioc"] = np.broadcast_to(
        (np.arange(E, dtype=f32) * c["CLOC"] - 1.0)[None, :], (128, E)).copy()

    in_maps = []
    for core in range(NC):
        m = dict(shared)
        lo = core * VC
        m["embrows"] = emb  # replicated full table
        loc = tokens[core * (len(tokens) // NC):(core + 1) * (len(tokens) // NC)]
        m["tokidx"] = np.ascontiguousarray(
            loc.reshape(-1, 128).T.astype(np.int32))  # [128, TC/128]
        esl = np.zeros((VCP, D), f32)
        esl[:VC] = emb[lo:lo + VC]
        m["embT"] = lhsT_tiles(esl, bf16=True)
        for l in range(L):
            m[f"w1{l}"] = lhsT_tiles(np.asarray(inputs["moe_w1"][l, core], f32), bf16=False)
            m[f"b1{l}"] = pp_cols(np.asarray(inputs["moe_b1"][l, core], f32))
            m[f"w2{l}"] = lhsT_tiles(np.asarray(inputs["moe_w2"][l, core], f32), bf16=False)
            m[f"b2{l}"] = pp_cols(np.asarray(inputs["moe_b2"][l, core], f32))
        in_maps.append(m)
    return in_maps


def assemble_logits(results, cfg):
    c = derived(cfg)
    B, S, V, VC = c["B"], c["S"], c["V"], c["VC"]
    out = np.empty((B, S, V), np.float32)
    for core, r in enumerate(results):
        lg = r["logits"]  # [VC, T]
        out[:, :, core * VC:(core + 1) * VC] = lg.T.reshape(B, S, VC)
    return out


# ---------------------------------------------------------------- device code

def build_nc(cfg):
    c = derived(cfg)
    L, D, E = c["L"], c["D"], c["E"]
    DK, FK = c["DK"], c["FK"]
    VC, VCK = c["VC"], c["VCK"]
    T = c["T"]

    nc = bacc.Bacc(None)
    P = {}

    def par(name, shape, dt):
        P[name] = nc.dram_tensor(name, shape, dt, kind="ExternalInput")

    par("tokidx", [128, T // (8 * 128)], I32)
    par("ones_mat", [128, 128], F32R)
    par("lt_mat", [128, 128], F32R)
    par("ioc", [128, E], F32)
    par("embrows", [c["V"], D], F32)
    par("embT", [VCK, 128, DK, 128], BF16)
    for l in range(L):
        par(f"wqk{l}", [2 * DK, 128, DK, 128], F32R)
        par(f"bqk{l}", [128, 2 * DK], F32)
        par(f"wv{l}", [DK, 128, D], F32R)
        par(f"bv{l}", [1, D], F32R)
        par(f"wo{l}", [DK, 128, DK, 128], F32R)
        par(f"bo{l}", [128, DK], F32)
        for nm in ("ln1w", "ln1b", "ln2w", "ln2b"):
            par(f"{nm}{l}", [128, DK], F32)
        par(f"wg{l}", [FK, 128, DK, 128], F32R)
        par(f"wu{l}", [FK, 128, DK, 128], F32R)
        par(f"wd{l}", [DK, 128, FK, 128], F32R)
        par(f"gw{l}", [DK, 128, E], F32)
        par(f"gb{l}", [1, E], F32R)
        par(f"w1{l}", [FK, 128, DK, 128], F32R)
        par(f"b1{l}", [128, FK], F32)
        par(f"w2{l}", [DK, 128, FK, 128], F32R)
        par(f"b2{l}", [128, DK], F32)
    par("rmsw", [128, DK], F32)
    OUT = nc.dram_tensor("logits", [VC, T], F32, kind="ExternalOutput")

    with tile.TileContext(nc) as tc:
        _emit(nc, tc, P, OUT, c)
    nc.compile()
    return nc


def _emit(nc, tc, P, OUT, c):
    NC, L, D, H, F, E = c["NC"], c["L"], c["D"], c["H"], c["F"], c["E"]
    TC, TCH, DK, FK = c["TC"], c["TCH"], c["DK"], c["FK"]
    VC, VCK, VS, SLOTS = c["VC"], c["VCK"], c["VS"], c["SLOTS"]
    G, T = c["G"], c["T"]
    CLOC, RSTG, CCH = c["CLOC"], c["RSTG"], c["CCH"]
    KCH = G * TCH
    NDN = max(1, D // 512)
    NW = min(512, D)
    GRP_KV = [list(range(g * G, (g + 1) * G)) for g in range(NC // G)]
    GRP_ALL = [list(range(NC))]

    from contextlib import ExitStack
    es = ExitStack()
    cst = es.enter_context(tc.tile_pool(name="cst", bufs=1))
    sbt = es.enter_context(tc.tile_pool(name="sbt", bufs=2))
    lnp = es.enter_context(tc.tile_pool(name="lnp", bufs=2))
    xlp = es.enter_context(tc.tile_pool(name="xlp", bufs=1))
    psm = es.enter_context(tc.tile_pool(name="psm", bufs=4, space="PSUM"))
    pst = es.enter_context(tc.tile_pool(name="pst", bufs=2, space="PSUM"))
    ptr = es.enter_context(tc.tile_pool(name="ptr", bufs=2, space="PSUM"))
    drp = es.enter_context(tc.tile_pool(name="drp", bufs=1, space="DRAM"))

    dbg_on = c.get("debug", False)

    def dbg(name, ap):
        if not dbg_on:
            return
        t = nc.dram_tensor(f"dbg_{name}", list(ap.shape), ap.dtype,
                           kind="ExternalOutput")
        nc.sync.dma_start(t[:], ap)

    ident = cst.tile([128, 128], F32, name="ident")
    make_identity(nc, ident)
    identr = cst.tile([128, 128], F32R, name="identr")
    nc.vector.tensor_copy(identr[:], ident[:])
    identb = cst.tile([128, 128], BF16, name="identb")
    nc.vector.tensor_copy(identb[:], ident[:])
    ones_m = cst.tile([128, 128], F32R, name="ones_m")
    nc.sync.dma_start(ones_m[:], P["ones_mat"][:])
    lt_sb = cst.tile([128, 128], F32R, name="lt_sb")
    nc.sync.dma_start(lt_sb[:], P["lt_mat"][:])
    ioc_sb = cst.tile([128, E], F32, name="ioc_sb")
    nc.sync.dma_start(ioc_sb[:], P["ioc"][:])
    rstgc = cst.tile([128, 1], F32, name="rstgc")
    nc.gpsimd.memset(rstgc[:], float(RSTG))
    clocc = cst.tile([128, 1], F32, name="clocc")
    nc.gpsimd.memset(clocc[:], float(CLOC))
    eps5 = cst.tile([128, 1], F32, name="eps5")
    nc.gpsimd.memset(eps5[:], 1e-5)
    eps6 = cst.tile([128, 1], F32, name="eps6")
    nc.gpsimd.memset(eps6[:], 1e-6)
    xT = cst.tile([128, DK, TC], F32R, name="xT")

    # routing state (per layer, reused buffers)
    offs1 = cst.tile([128, TCH], I32, name="offs1")
    offs2 = cst.tile([128, TCH], I32, name="offs2")
    wts1 = cst.tile([128, TCH], F32, name="wts1")
    wts2 = cst.tile([128, TCH], F32, name="wts2")
    base_rep = cst.tile([128, E], F32, name="base_rep")

    # MoE staging DRAM buffers (shared across layers)
    disp_stage = drp.tile([RSTG, D], BF16, name="dstg", tag="dstg")
    disp_recv = drp.tile([RSTG, D], BF16, name="drcv", tag="drcv")
    ret_stage = drp.tile([RSTG, D], BF16, name="rstg", tag="rstg")
    ret_recv = drp.tile([RSTG + 128, D], BF16, name="rrcv", tag="rrcv")
    # zero row at RSTG: dropped tokens gather zeros
    zrowf = cst.tile([1, D], F32, name="zrowf")
    nc.gpsimd.memset(zrowf[:], 0.0)
    zrow = cst.tile([1, D], BF16, name="zrow")
    nc.vector.tensor_copy(zrow[:], zrowf[:])
    nc.sync.dma_start(ret_recv[RSTG:RSTG + 1, :], zrow[:])

    # ---------------- embedding: gather own tokens from replicated table
    with tc.tile_pool(name="emb_ph", bufs=3) as ph:
        idx_sb = ph.tile([128, TCH], I32, name="idx_sb", bufs=1)
        nc.sync.dma_start(idx_sb[:], P["tokidx"][:])
        sqrt_d = float(np.sqrt(c["D"]))
        for tm in range(TCH):
            ge = ph.tile([128, D], F32, tag="ge")
            nc.gpsimd.indirect_dma_start(
                out=ge[:], out_offset=None, in_=P["embrows"][:],
                in_offset=bass.IndirectOffsetOnAxis(ap=idx_sb[:, tm:tm + 1], axis=0))
            for k in range(DK):
                pt = ptr.tile([128, 128], F32, tag="ptr")
                nc.tensor.transpose(pt[:], ge[:, k * 128:(k + 1) * 128], ident[:])
                nc.scalar.activation(xT[:, k, tm * 128:(tm + 1) * 128], pt[:],
                                     AF.Copy, scale=sqrt_d)
    dbg("x0T", xT[:])

    # ---------------- LN helper (matmul stats, replicated across partitions)
    def layer_norm_(dst, src, wcols, bcols, eps, skip_mean=False):
        eps = eps5[:, 0:1] if eps == 1e-5 else eps6[:, 0:1]
        ps1 = None if skip_mean else pst.tile([128, TC], F32, tag="pstat")
        ps2 = pst.tile([128, TC], F32, tag="pstat")
        for k in range(DK):
            sq = lnp.tile([128, TC], F32R, tag="sq")
            nc.vector.tensor_tensor(sq[:], src[:, k, :], src[:, k, :], OP.mult)
            if not skip_mean:
                nc.tensor.matmul(ps1[:], ones_m[:], src[:, k, :],
                                 start=(k == 0), stop=(k == DK - 1))
            nc.tensor.matmul(ps2[:], ones_m[:], sq[:],
                             start=(k == 0), stop=(k == DK - 1))
        e2 = lnp.tile([128, TC], F32, tag="stmp")
        nc.scalar.activation(e2[:], ps2[:], AF.Copy, scale=1.0 / c["D"])
        if not skip_mean:
            mu = lnp.tile([128, TC], F32, tag="smu", bufs=1)
            nc.scalar.activation(mu[:], ps1[:], AF.Copy, scale=1.0 / c["D"])
            var = lnp.tile([128, TC], F32, tag="stmp")
            nc.vector.tensor_tensor(var[:], mu[:], mu[:], OP.mult)
            nc.vector.tensor_tensor(var[:], e2[:], var[:], OP.subtract)
        else:
            var = e2
        sd = lnp.tile([128, TC], F32, tag="stmp")
        nc.scalar.activation(sd[:], var[:], AF.Sqrt, bias=eps)
        rstd = lnp.tile([128, TC], F32, tag="srstd", bufs=1)
        nc.vector.reciprocal(rstd[:], sd[:])
        for k in range(DK):
            t1 = lnp.tile([128, TC], F32, tag="lnt")
            if not skip_mean:
                nc.vector.tensor_tensor(t1[:], src[:, k, :], mu[:], OP.subtract)
                nc.vector.tensor_tensor(t1[:], t1[:], rstd[:], OP.mult)
            else:
                nc.vector.tensor_tensor(t1[:], src[:, k, :], rstd[:], OP.mult)
            if bcols is not None:
                nc.vector.tensor_scalar(dst[:, k, :], t1[:],
                                        wcols[:, k:k + 1], bcols[:, k:k + 1],
                                        OP.mult, OP.add)
            else:
                nc.vector.tensor_scalar_mul(dst[:, k, :], t1[:], wcols[:, k:k + 1])

    # ---------------- layers
    for l in range(L):
        lb = {}
        for nm in ("bqk", "bo", "ln1w", "ln1b", "ln2w", "ln2b", "b1", "b2"):
            w = P[f"{nm}{l}"].shape[1]
            t = cst.tile([128, w], F32, name=f"{nm}{l}_sb", tag=f"c_{nm}")
            nc.sync.dma_start(t[:], P[f"{nm}{l}"][:])
            lb[nm] = t
        bv1 = cst.tile([1, D], F32R, name=f"bv1_{l}", tag="c_bv1")
        nc.sync.dma_start(bv1[:], P[f"bv{l}"][:])
        bv = cst.tile([128, D], F32, name=f"bv{l}_sb", tag="c_bv")
        for dn in range(NDN):
            psb = psm.tile([128, NW], F32, tag="psmm")
            nc.tensor.matmul(psb[:], ones_m[0:1, :],
                             bv1[0:1, dn * NW:(dn + 1) * NW], start=True, stop=True)
            nc.vector.tensor_copy(bv[:, dn * NW:(dn + 1) * NW], psb[:])
        gb1 = cst.tile([1, E], F32R, name=f"gb1_{l}", tag="c_gb1")
        nc.sync.dma_start(gb1[:], P[f"gb{l}"][:])
        psgb = psm.tile([128, E], F32, tag="psmm")
        nc.tensor.matmul(psgb[:], ones_m[0:1, :], gb1[0:1, :], start=True, stop=True)
        gb = cst.tile([128, E], F32, name=f"gb{l}_sb", tag="c_gb")
        nc.vector.tensor_copy(gb[:], psgb[:])

        kv_in = [drp.tile([128, TC], F32R, name=f"kvin{sl}", tag=f"kvin{sl}")
                 for sl in range(SLOTS)]
        kv_all = [drp.tile([G, 128, TC], F32R, name=f"kvall{sl}", tag=f"kvall{sl}")
                  for sl in range(SLOTS)]

        # --- qkv phase (v first, then q/k with chunked k gathers)
        with tc.tile_pool(name="qp", bufs=1) as qp:
            q_sb = qp.tile([128, DK, TC], F32R, tag="q_sb")
            with (
                tc.tile_pool(name="qphw", bufs=4) as qphw,
                tc.tile_pool(name="qphk", bufs=2) as qphk,
                tc.tile_pool(name="qpv", bufs=1) as qpv,
            ):
                for dn in range(NDN):
                    wv = qpv.tile([128, DK, NW], F32R, tag="wv")
                    for k in range(DK):
                        nc.sync.dma_start(wv[:, k, :],
                                          P[f"wv{l}"][k, :, dn * NW:(dn + 1) * NW])
                    for tm in range(TCH):
                        ps = psm.tile([128, NW], F32, tag="psmm")
                        for k in range(DK):
                            nc.tensor.matmul(ps[:], xT[:, k, tm * 128:(tm + 1) * 128],
                                             wv[:, k, :],
                                             start=(k == 0), stop=(k == DK - 1))
                        vt = qphk.tile([128, NW], F32R, tag="vt")
                        nc.vector.tensor_tensor(
                            vt[:], ps[:], bv[:, dn * NW:(dn + 1) * NW], OP.add)
                        for sl in range(max(1, NW // TC)):
                            w_ = min(TC, NW)
                            slot = DK + tm * VS + (dn * NW) // TC + sl
                            nc.sync.dma_start(kv_in[slot][:],
                                              vt[:, sl * w_:(sl + 1) * w_])
                            nc.gpsimd.collective_compute(
                                "AllGather", OP.bypass, replica_groups=GRP_KV,
                                ins=[kv_in[slot][:]], outs=[kv_all[slot][:]])
                for m in range(2 * DK):
                    wt = qphw.tile([128, DK, 128], F32R, tag="wt")
                    nc.sync.dma_start(wt[:], P[f"wqk{l}"][m])
                    ps = psm.tile([128, TC], F32, tag="psmm")
                    for k in range(DK):
                        nc.tensor.matmul(ps[:], wt[:, k, :], xT[:, k, :],
                                         start=(k == 0), stop=(k == DK - 1))
                    if m < DK:
                        nc.scalar.activation(q_sb[:, m, :], ps[:], AF.Identity,
                                             scale=0.125, bias=lb["bqk"][:, m:m + 1])
                    else:
                        kt = qphk.tile([128, TC], F32R, tag="kt")
                        nc.scalar.activation(kt[:], ps[:], AF.Identity,
                                             bias=lb["bqk"][:, m:m + 1])
                        nc.sync.dma_start(kv_in[m - DK][:], kt[:])
                        nc.gpsimd.collective_compute(
                            "AllGather", OP.bypass, replica_groups=GRP_KV,
                            ins=[kv_in[m - DK][:]], outs=[kv_all[m - DK][:]])
            if l == 0:
                dbg("q0", q_sb[:])

            # --- attention (q_sb in scope)
            with tc.tile_pool(name="aoT", bufs=1) as aoTp:
                oT = aoTp.tile([128, DK, TC], F32R, tag="oT")
                vh2 = aoTp.tile([128, 2, KCH, 128], F32R, tag="vh2")
                for b_ in range(2):
                    for kc_ in range(KCH):
                        nc.sync.dma_start(vh2[:, b_, kc_, 64:128],
                                          P["ones_mat"][:, 0:64])
                with (
                    tc.tile_pool(name="aph", bufs=2) as aph,
                    tc.tile_pool(name="apT", bufs=1) as apTp,
                ):
                    for h in range(H):
                        qm, qoff = h // 2, 64 * (h % 2)
                        kh = aph.tile([128, G, TC], F32R, tag="kh")
                        for g in range(G):
                            nc.sync.dma_start(kh[qoff:qoff + 64, g, :],
                                              kv_all[qm][g, qoff:qoff + 64, :])
                        s_v, off_v = (64 * h) // TC, (64 * h) % TC
                        for g in range(G):
                            for tm in range(TCH):
                                nc.sync.dma_start(
                                    vh2[:, h % 2, g * TCH + tm, 0:64],
                                    kv_all[DK + tm * VS + s_v][g, :, off_v:off_v + 64])
                        pT = apTp.tile([128, KCH, TC], F32R, tag="pT")
                        for kc in range(KCH):
                            ps = psm.tile([128, TC], F32, tag="psmm")
                            nc.tensor.matmul(
                                ps[:],
                                kh[qoff:qoff + 64, kc // TCH,
                                   (kc % TCH) * 128:(kc % TCH) * 128 + 128],
                                q_sb[qoff:qoff + 64, qm, :], start=True, stop=True)
                            nc.scalar.activation(pT[:, kc, :], ps[:], AF.Exp)
                        po = psm.tile([128, TC], F32, tag="psmm")
                        for kc in range(KCH):
                            nc.tensor.matmul(po[:], vh2[:, h % 2, kc, :],
                                             pT[:, kc, :],
                                             start=(kc == 0), stop=(kc == KCH - 1))
                        rec = sbt.tile([64, TC], F32, tag="rec")
                        nc.vector.reciprocal(rec[:], po[64:128, :])
                        nc.vector.tensor_tensor(oT[qoff:qoff + 64, qm, :],
                                                po[0:64, :], rec[:], OP.mult)
                if l == 0:
                    dbg("oT0", oT[:])
                # --- out proj + residual + ln1
                with tc.tile_pool(name="oph", bufs=4) as oph:
                    xln = xlp.tile([128, DK, TC], F32R, tag="xln")
                    for m in range(DK):
                        wt = oph.tile([128, DK, 128], F32R, tag="wt")
                        nc.sync.dma_start(wt[:], P[f"wo{l}"][m])
                        ps = psm.tile([128, TC], F32, tag="psmm")
                        for k in range(DK):
                            nc.tensor.matmul(ps[:], wt[:, k, :], oT[:, k, :],
                                             start=(k == 0), stop=(k == DK - 1))
                        t = sbt.tile([128, TC], F32, tag="ot")
                        nc.vector.tensor_scalar_add(t[:], ps[:], lb["bo"][:, m:m + 1])
                        nc.vector.tensor_tensor(xln[:, m, :], t[:], xT[:, m, :],
                                                OP.add)
                    layer_norm_(xT, xln, lb["ln1w"], lb["ln1b"], 1e-5)
        if l == 0:
            dbg("xln1_0", xT[:])

        # --- router: gate -> top2 -> per-(expert) positions -> scatter rows
        nc.gpsimd.memset(base_rep[:], 0.0)
        with tc.tile_pool(name="rph", bufs=2) as rph:
            gwt = rph.tile([128, DK, E], F32, tag="gwt", bufs=1)
            for k in range(DK):
                nc.sync.dma_start(gwt[:, k, :], P[f"gw{l}"][k])
            for tm in range(TCH):
                xf = rph.tile([128, DK, 128], F32, tag="xf")
                for k in range(DK):
                    nc.vector.tensor_copy(xf[:, k, :],
                                          xT[:, k, tm * 128:(tm + 1) * 128])
                psg = psm.tile([128, E], F32, tag="psmm")
                for k in range(DK):
                    nc.tensor.matmul(psg[:], xf[:, k, :], gwt[:, k, :],
                                     start=(k == 0), stop=(k == DK - 1))
                gs = rph.tile([128, E], F32, tag="gs")
                nc.vector.tensor_tensor(gs[:], psg[:], gb[:], OP.add)
                m1 = rph.tile([128, 1], F32, tag="m1")
                nc.vector.tensor_reduce(m1[:], gs[:], AX.X, OP.max)
                mask1 = rph.tile([128, E], F32, tag="mask1")
                nc.vector.tensor_tensor(mask1[:], gs[:],
                                        m1[:].to_broadcast([128, E]), OP.is_equal)
                gs2 = rph.tile([128, E], F32, tag="gs2")
                nc.vector.tensor_scalar_mul(gs2[:], mask1[:], -1e30)
                nc.vector.tensor_tensor(gs2[:], gs2[:], gs[:], OP.add)
                m2 = rph.tile([128, 1], F32, tag="m2")
                nc.vector.tensor_reduce(m2[:], gs2[:], AX.X, OP.max)
                mask2 = rph.tile([128, E], F32, tag="mask2")
                nc.vector.tensor_tensor(mask2[:], gs2[:],
                                        m2[:].to_broadcast([128, E]), OP.is_equal)
                dm = rph.tile([128, 1], F32, tag="dm")
                nc.vector.tensor_tensor(dm[:], m2[:], m1[:], OP.subtract)
                nc.scalar.activation(dm[:], dm[:], AF.Exp)
                nc.vector.tensor_scalar_add(dm[:], dm[:], 1.0)
                nc.vector.reciprocal(wts1[:, tm:tm + 1], dm[:])
                nc.vector.tensor_scalar(wts2[:, tm:tm + 1], wts1[:, tm:tm + 1],
                                        -1.0, 1.0, OP.mult, OP.add)
                # --- routing positions
                ind = rph.tile([128, E], F32R, tag="ind")
                nc.vector.tensor_tensor(ind[:], mask1[:], mask2[:], OP.add)
                psc = psm.tile([128, E], F32, tag="psmm")
                nc.tensor.matmul(psc[:], lt_sb[:], ind[:], start=True, stop=True)
                # incl = base + cumsum (1-based local position)
                incl = rph.tile([128, E], F32, tag="incl")
                nc.vector.tensor_tensor(incl[:], psc[:], base_rep[:], OP.add)
                post = rph.tile([128, E], F32, tag="post")
                nc.vector.tensor_tensor(post[:], incl[:], ioc_sb[:], OP.add)
                ovfm = rph.tile([128, E], F32, tag="ovfm")
                nc.vector.tensor_tensor(ovfm[:], incl[:],
                                        clocc[:].to_broadcast([128, E]),
                                        OP.is_gt)
                for mask, offs in ((mask1, offs1), (mask2, offs2)):
                    tmp = rph.tile([128, E], F32, tag="tmp")
                    nc.vector.tensor_tensor(tmp[:], mask[:], post[:], OP.mult)
                    offf = rph.tile([128, 1], F32, tag="offf")
                    nc.vector.tensor_reduce(offf[:], tmp[:], AX.X, OP.add)
                    nc.vector.tensor_tensor(tmp[:], mask[:], ovfm[:], OP.mult)
                    ovr = rph.tile([128, 1], F32, tag="ovr")
                    nc.vector.tensor_reduce(ovr[:], tmp[:], AX.X, OP.add)
                    offg = rph.tile([128, 1], F32, tag="offg")
                    nc.vector.tensor_tensor(offg[:], rstgc[:], offf[:],
                                            OP.subtract)
                    nc.vector.tensor_tensor(offg[:], offg[:], ovr[:], OP.mult)
                    nc.vector.tensor_tensor(offg[:], offg[:], offf[:], OP.add)
                    nc.vector.tensor_copy(offs[:, tm:tm + 1], offg[:])
                # update base with this chunk's totals (colsum, replicated)
                psb = psm.tile([128, E], F32, tag="psmm")
                nc.tensor.matmul(psb[:], ones_m[:], ind[:], start=True, stop=True)
                nc.vector.tensor_tensor(base_rep[:], base_rep[:], psb[:], OP.add)
                # --- transpose x rows (token-major, bf16) and scatter
                xtb = rph.tile([128, D], BF16, tag="xtb")
                for k in range(DK):
                    pt = ptr.tile([128, 128], F32R, tag="ptr")
                    nc.tensor.transpose(pt[:], xT[:, k, tm * 128:(tm + 1) * 128],
                                        identr[:])
                    nc.scalar.activation(xtb[:, k * 128:(k + 1) * 128], pt[:],
                                         AF.Copy)
                for offs in (offs1, offs2):
                    nc.gpsimd.indirect_dma_start(
                        out=disp_stage[:],
                        out_offset=bass.IndirectOffsetOnAxis(
                            ap=offs[:, tm:tm + 1], axis=0),
                        in_=xtb[:], in_offset=None,
                        bounds_check=RSTG - 1, oob_is_err=False)
        nc.gpsimd.collective_compute(
            "AllToAll", OP.bypass, replica_groups=GRP_ALL,
            ins=[disp_stage[:]], outs=[disp_recv[:]])

        # --- ds mlp (local tokens)
        dsT_pool = tc.tile_pool(name="dsT", bufs=1)
        dsTp = dsT_pool.__enter__()
        dsT = dsTp.tile([128, DK, TC], F32, tag="dsT")
        with (
            tc.tile_pool(name="dph", bufs=1) as dph,
            tc.tile_pool(name="dphw", bufs=3) as dphw,
        ):
            gu = dph.tile([128, FK, TC], F32R, tag="gu")
            for m in range(FK):
                wtg = dphw.tile([128, DK, 128], F32R, tag="wt")
                nc.sync.dma_start(wtg[:], P[f"wg{l}"][m])
                psg = psm.tile([128, TC], F32, tag="psmm")
                for k in range(DK):
                    nc.tensor.matmul(psg[:], wtg[:, k, :], xT[:, k, :],
                                     start=(k == 0), stop=(k == DK - 1))
                sg = sbt.tile([128, TC], F32, tag="sg")
                nc.scalar.activation(sg[:], psg[:], AF.Sigmoid)
                nc.vector.tensor_tensor(sg[:], sg[:], psg[:], OP.mult)
                wtu = dphw.tile([128, DK, 128], F32R, tag="wt")
                nc.sync.dma_start(wtu[:], P[f"wu{l}"][m])
                psu = psm.tile([128, TC], F32, tag="psmm")
                for k in range(DK):
                    nc.tensor.matmul(psu[:], wtu[:, k, :], xT[:, k, :],
                                     start=(k == 0), stop=(k == DK - 1))
                nc.vector.tensor_tensor(gu[:, m, :], sg[:], psu[:], OP.mult)
            for m in range(DK):
                wtd = dphw.tile([128, FK, 128], F32R, tag="wtd", bufs=2)
                nc.sync.dma_start(wtd[:], P[f"wd{l}"][m])
                psd = psm.tile([128, TC], F32, tag="psmm")
                for k in range(FK):
                    nc.tensor.matmul(psd[:], wtd[:, k, :], gu[:, k, :],
                                     start=(k == 0), stop=(k == FK - 1))
                nc.vector.tensor_copy(dsT[:, m, :], psd[:])
        if l == 0:
            dbg("dsT0", dsT[:])

        # --- expert pass over this core's routed rows (<= RSTG of them)
        # f32r throughout (bf16 only on the wire): w1/w2 tiles streamed per
        # column chunk, eo computed d-major then transposed to token rows.
        with (
            tc.tile_pool(name="ewq", bufs=3) as ewq,
            tc.tile_pool(name="ewd", bufs=2) as ewd,
            tc.tile_pool(name="eph", bufs=1) as eph,
            tc.tile_pool(name="ehp", bufs=1) as ehp,
            tc.tile_pool(name="erp", bufs=2) as erp,
        ):
            for cc in range(CCH):
                rt = eph.tile([128, 4, D], BF16, tag="rt", bufs=1)
                for j in range(4):
                    nc.sync.dma_start(
                        rt[:, j, :],
                        disp_recv[cc * 512 + j * 128:cc * 512 + (j + 1) * 128, :])
                xeT = eph.tile([128, DK, 512], F32R, tag="xeT", bufs=1)
                for j in range(4):
                    for k in range(DK):
                        pt = ptr.tile([128, 128], BF16, tag="ptr")
                        nc.tensor.transpose(pt[:], rt[:, j, k * 128:(k + 1) * 128],
                                            identb[:])
                        nc.scalar.activation(xeT[:, k, j * 128:(j + 1) * 128],
                                             pt[:], AF.Copy)
                hsb = ehp.tile([128, FK, 512], F32R, tag="hsb")
                for m in range(FK):
                    wt1 = ewq.tile([128, DK, 128], F32R, tag="wt1")
                    nc.sync.dma_start(wt1[:], P[f"w1{l}"][m])
                    hps = psm.tile([128, 512], F32, tag="psmm")
                    for k in range(DK):
                        nc.tensor.matmul(hps[:], wt1[:, k, :], xeT[:, k, :],
                                         start=(k == 0), stop=(k == DK - 1))
                    nc.scalar.activation(hsb[:, m, :], hps[:], AF.Relu,
                                         bias=lb["b1"][:, m:m + 1])
                rra = erp.tile([128, 4, D], BF16, tag="rra", bufs=1)
                for m in range(DK):
                    wt2 = ewd.tile([128, FK, 128], F32R, tag="wt2")
                    nc.sync.dma_start(wt2[:], P[f"w2{l}"][m])
                    eps_ = psm.tile([128, 512], F32, tag="psmm")
                    for k in range(FK):
                        nc.tensor.matmul(eps_[:], wt2[:, k, :], hsb[:, k, :],
                                         start=(k == 0), stop=(k == FK - 1))
                    eod = erp.tile([128, 512], F32, tag="eod")
                    nc.vector.tensor_scalar_add(eod[:], eps_[:],
                                                lb["b2"][:, m:m + 1])
                    for j in range(4):
                        pt = ptr.tile([128, 128], F32, tag="ptr")
                        nc.tensor.transpose(pt[:], eod[:, j * 128:(j + 1) * 128],
                                            ident[:])
                        nc.scalar.activation(rra[:, j, m * 128:(m + 1) * 128],
                                             pt[:], AF.Copy)
                for j in range(4):
                    nc.sync.dma_start(
                        ret_stage[cc * 512 + j * 128:cc * 512 + (j + 1) * 128, :],
                        rra[:, j, :])
        nc.gpsimd.collective_compute(
            "AllToAll", OP.bypass, replica_groups=GRP_ALL,
            ins=[ret_stage[:]], outs=[ret_recv[0:RSTG, :]])

        # --- combine: gather own tokens' two expert rows, weight, + ds, ln2
        xln2 = xlp.tile([128, DK, TC], F32R, tag="xln")
        with tc.tile_pool(name="cph", bufs=2) as cph:
            for tm in range(TCH):
                g1 = cph.tile([128, D], BF16, tag="g1")
                nc.gpsimd.indirect_dma_start(
                    out=g1[:], out_offset=None, in_=ret_recv[:],
                    in_offset=bass.IndirectOffsetOnAxis(
                        ap=offs1[:, tm:tm + 1], axis=0))
                g2 = cph.tile([128, D], BF16, tag="g2")
                nc.gpsimd.indirect_dma_start(
                    out=g2[:], out_offset=None, in_=ret_recv[:],
                    in_offset=bass.IndirectOffsetOnAxis(
                        ap=offs2[:, tm:tm + 1], axis=0))
                mo = cph.tile([128, D], F32R, tag="mo")
                mo2 = cph.tile([128, D], F32R, tag="mo2")
                nc.vector.tensor_scalar_mul(mo[:], g1[:], wts1[:, tm:tm + 1])
                nc.vector.tensor_scalar_mul(mo2[:], g2[:], wts2[:, tm:tm + 1])
                nc.vector.tensor_tensor(mo[:], mo[:], mo2[:], OP.add)
                for k in range(DK):
                    pt = ptr.tile([128, 128], F32R, tag="ptr")
                    nc.tensor.transpose(pt[:], mo[:, k * 128:(k + 1) * 128],
                                        identr[:])
                    tcb = cph.tile([128, 128], F32, tag="tcb")
                    nc.vector.tensor_tensor(tcb[:], pt[:],
                                            dsT[:, k, tm * 128:(tm + 1) * 128],
                                            OP.add)
                    nc.vector.tensor_scalar_mul(tcb[:], tcb[:], 0.5)
                    nc.vector.tensor_tensor(
                        xln2[:, k, tm * 128:(tm + 1) * 128], tcb[:],
                        xT[:, k, tm * 128:(tm + 1) * 128], OP.add)
        layer_norm_(xT, xln2, lb["ln2w"], lb["ln2b"], 1e-5)
        dsT_pool.__exit__(None, None, None)

    dbg("xfinal", xT[:])
    # ---------------- final rms + allgather + lm_head (weight-stationary)
    rmsw = cst.tile([128, DK], F32, name="rmsw_sb")
    nc.sync.dma_start(rmsw[:], P["rmsw"][:])
    xf_in = drp.tile([128, DK, TC], BF16, name="xfin")
    xf_all = drp.tile([NC, 128, DK, TC], BF16, name="xfall", addr_space="Shared")
    xr = xlp.tile([128, DK, TC], F32R, tag="xln")
    layer_norm_(xr, xT, rmsw, None, 1e-6, skip_mean=True)
    xrb = xlp.tile([128, DK, TC], BF16, tag="xrb")
    for k in range(DK):
        nc.vector.tensor_copy(xrb[:, k, :], xr[:, k, :])
    nc.sync.dma_start(xf_in[:], xrb[:])
    nc.gpsimd.collective_compute(
        "AllGather", OP.bypass, replica_groups=GRP_ALL,
        ins=[xf_in[:]], outs=[xf_all[:]])
    with (
        tc.tile_pool(name="lph", bufs=1) as lph,
        tc.tile_pool(name="lphw", bufs=3) as lphw,
    ):
        xfs = lph.tile([128, NC, DK, TC], BF16, tag="xfs")
        for n in range(NC):
            nc.sync.dma_start(xfs[:, n], xf_all[n])
        for m in range(VCK):
            wt = lphw.tile([128, DK, 128], BF16, tag="wt")
            nc.sync.dma_start(wt[:], P["embT"][m])
            rows = min(128, VC - m * 128)
            for n in range(NC):
                ps = psm.tile([128, TC], F32, tag="psmm")
                for k in range(DK):
                    nc.tensor.matmul(ps[:], wt[:, k, :], xfs[:, n, k, :],
                                     start=(k == 0), stop=(k == DK - 1))
                lo = sbt.tile([128, TC], F32, tag="lo")
                nc.vector.tensor_copy(lo[:], ps[:])
                nc.sync.dma_start(
                    OUT[m * 128:m * 128 + rows, n * TC:(n + 1) * TC], lo[:rows, :])

    es.close()


# ---------------------------------------------------------------- runner

def run_model(inputs, cfg, nc=None):
    c = derived(cfg)
    in_maps = prep_in_maps(inputs, cfg)
    if nc is None:
        nc = build_nc(cfg)
    res = run_bass_kernel_spmd(nc, in_maps, core_ids=list(range(c["NC"])))
    return assemble_logits(res.results, cfg), nc


# ---------------------------------------------------------------- entry point

_NC_CACHE = None


def kernel(**inputs):
    """Full-model forward on 8 trn2 cores. inputs as in reference.setup_inputs()."""
    global _NC_CACHE
    import numpy as _np
    inputs = {k: _np.asarray(v) for k, v in inputs.items()}
    if _NC_CACHE is None:
        _NC_CACHE = build_nc(FULL_CFG)
    in_maps = prep_in_maps(inputs, FULL_CFG)
    res = run_bass_kernel_spmd(_NC_CACHE, in_maps,
                               core_ids=list(range(FULL_CFG["NC"])))
    return assemble_logits(res.results, FULL_CFG)
